# revision 23
# baseline (speedup 1.0000x reference)
"""Trainium2 Bass kernel for DetectionLayer (refine + per-class NMS).

Contract: kernel(rois, probs, deltas) with FULL inputs
  rois   [16, 4096, 4]   f32
  probs  [16, 4096, 81]  f32
  deltas [16, 4096, 81, 4] f32
returns [16, 100, 6] f32 detections, matching the jax reference.

Sharding: pure data parallel - 2 images per core across 8 NeuronCores.

Structure:
  Fast phase (always): load probs for both images in a fused
  [128, 64, 81] layout as two full-partition chunks, one per HWDGE
  queue (SDMA engines interleave both queues' packets, hiding
  per-descriptor HBM latency -> ~350 GB/s). Detect whether ANY roi
  passes min-confidence with the work split between DVE (reduce_max +
  is_ge count) and the Scalar engine (Relu(p - (conf - eps)) with
  sum-accum), reduce via a PE ones-matmul, and write a zeroed output
  early (overlaps the DMA).
  Guard: a single tc.If(any > 0) branch (a branch, not a loop skip --
  no back-edge barriers).
  Slow phase (inside the If; only when detections exist): per-image
  probs/rois/deltas load, argmax-class delta select, box refine,
  per-class NMS via the class-offset trick, rewrite of the output.

Known limitation (inherited from the original kernel, NMS loop kept
bit-identical): when two rois have exactly equal f32 class scores, the
masked-sum row extraction in the NMS loop merges both tied rows (the
reference argmax keeps only the first). Only reachable on data with
exact score ties among detections above min-confidence.
"""

import os as _os

import numpy as np

import concourse.bacc as bacc
import concourse.bass as bass
import concourse.bass_isa as bass_isa
import concourse.mybir as mybir
from concourse.expressions import smin
from concourse.tile import TileContext

B = 16              # full batch
NCORES = 8
BPC = B // NCORES   # images per core
N = 4096            # rois per image
C = 81              # classes
K = 100             # detection_max_instances
P = 128             # SBUF partitions
NPF = BPC * N // P  # fused rois per partition (64); partition p -> image p//64
NPI = N // P        # rois per partition, per-image layout (32)
NEG = -1e9
MIN_CONF = 0.7
NMS_T = 0.3
F32 = mybir.dt.float32
I32 = mybir.dt.int32

# probs DMA chunk schedule: <engine><slots>,... with s=sync, a=scalar(ACT)
QCFG = _os.environ.get("DETK_Q", "s32,a32")


def _slow_image(nc, tc, pools, img, rois_t, probs_t, deltas_t, state, det):
    """Refine + NMS-state for one image (runs only inside the If guard).
    Re-loads probs in the per-image layout and recomputes scores there."""
    cpool, big, sm = pools
    crev = state["crev"]
    negs = state["negs"]
    cnt2 = state["cnt2"]

    probs_ap = probs_t[img].rearrange("(p n) c -> p n c", p=P)        # [128,32,81]
    rois_ap = rois_t[img].rearrange("(p n) k -> p n k", p=P)          # [128,32,4]
    deltas_ap = deltas_t[img].rearrange("(p n) c k -> p n c k", p=P)

    pt = big.tile([P, NPI, C], F32, tag=f"probs{img}")
    for s in range(4):
        sl = slice(32 * s, 32 * s + 32)
        nc.sync.dma_start(out=pt[sl], in_=probs_ap[sl])
    dt_ = big.tile([P, NPI, C, 4], F32, tag=f"deltas{img}")
    for s in range(8):
        sl = slice(16 * s, 16 * s + 16)
        nc.sync.dma_start(out=dt_[sl], in_=deltas_ap[sl])
    rt = sm.tile([P, NPI, 4], F32, tag=f"rois{img}")
    nc.sync.dma_start(out=rt, in_=rois_ap)

    scores = sm.tile([P, NPI], F32, tag=f"sc_s{img}")
    nc.vector.reduce_max(scores, pt, axis=mybir.AxisListType.X)
    ge = sm.tile([P, NPI], F32, tag=f"ge{img}")
    nc.vector.tensor_scalar(
        out=ge, in0=scores, scalar1=MIN_CONF, scalar2=None,
        op0=mybir.AluOpType.is_ge, op1=mybir.AluOpType.add,
        accum_out=cnt2[:, img : img + 1],
    )

    # NMS-state tiles read by the NMS loop
    sc = sm.tile([P, NPI], F32, tag=f"sc{img}")
    ob = sm.tile([P, NPI, 4], F32, tag=f"ob{img}")
    ar = sm.tile([P, NPI], F32, tag=f"ar{img}")
    cat = sm.tile([P, NPI, 6], F32, tag=f"cat{img}")
    mr = sm.tile([P, 8], F32, tag=f"mr{img}")

    # one-hot mask of argmax class: M = (probs == score), in place over probs
    m = pt
    nc.vector.tensor_tensor(
        m, pt, scores.unsqueeze(2).to_broadcast([P, NPI, C]),
        op=mybir.AluOpType.is_equal,
    )

    # select argmax-class delta: deltas *= M (bcast over k), sum over c
    d_perm = dt_.rearrange("p n c k -> p n k c")
    nc.vector.tensor_tensor(
        d_perm, d_perm, m.unsqueeze(2).to_broadcast([P, NPI, 4, C]),
        op=mybir.AluOpType.mult,
    )
    dsel = sm.tile([P, NPI, 4], F32, tag=f"dsel{img}")
    nc.vector.reduce_sum(dsel, d_perm, axis=mybir.AxisListType.X)

    # class id = 80 - max((80-c) * M)  (ties -> smallest c, like argmax)
    nc.vector.tensor_tensor(m, m, crev, op=mybir.AluOpType.mult)
    cid = sm.tile([P, NPI], F32, tag=f"cid{img}")
    nc.vector.reduce_max(cid, m, axis=mybir.AxisListType.X)
    nc.vector.tensor_scalar(
        out=cid, in0=cid, scalar1=-1.0, scalar2=float(C - 1),
        op0=mybir.AluOpType.mult, op1=mybir.AluOpType.add,
    )

    # bbox_std scaling (match reference op order exactly)
    nc.vector.tensor_scalar_mul(dsel[:, :, 0:2], dsel[:, :, 0:2], 0.1)
    nc.vector.tensor_scalar_mul(dsel[:, :, 2:4], dsel[:, :, 2:4], 0.2)

    # ---- apply deltas + clip (mirrors _apply_deltas fp32 op order) ----
    h = sm.tile([P, NPI], F32, tag=f"h{img}")
    w = sm.tile([P, NPI], F32, tag=f"w{img}")
    nc.vector.tensor_sub(h, rt[:, :, 2], rt[:, :, 0])
    nc.vector.tensor_sub(w, rt[:, :, 3], rt[:, :, 1])
    t1 = sm.tile([P, NPI], F32, tag=f"t1{img}")
    t2 = sm.tile([P, NPI], F32, tag=f"t2{img}")
    cy = sm.tile([P, NPI], F32, tag=f"cy{img}")
    cx = sm.tile([P, NPI], F32, tag=f"cx{img}")
    # cy = y1 + 0.5*h + dy*h
    nc.vector.tensor_scalar_mul(t1, h, 0.5)
    nc.vector.tensor_add(t2, rt[:, :, 0], t1)
    nc.vector.tensor_mul(t1, dsel[:, :, 0], h)
    nc.vector.tensor_add(cy, t2, t1)
    # cx = x1 + 0.5*w + dx*w
    nc.vector.tensor_scalar_mul(t1, w, 0.5)
    nc.vector.tensor_add(t2, rt[:, :, 1], t1)
    nc.vector.tensor_mul(t1, dsel[:, :, 1], w)
    nc.vector.tensor_add(cx, t2, t1)
    # h *= exp(dh); w *= exp(dw)
    e = sm.tile([P, NPI], F32, tag=f"e{img}")
    nc.scalar.activation(e, dsel[:, :, 2], mybir.ActivationFunctionType.Exp)
    nc.vector.tensor_mul(h, h, e)
    nc.scalar.activation(e, dsel[:, :, 3], mybir.ActivationFunctionType.Exp)
    nc.vector.tensor_mul(w, w, e)

    ref = sm.tile([P, NPI, 4], F32, tag=f"ref{img}")
    nc.vector.tensor_scalar_mul(t1, h, 0.5)
    nc.vector.tensor_sub(ref[:, :, 0], cy, t1)
    nc.vector.tensor_add(ref[:, :, 2], cy, t1)
    nc.vector.tensor_scalar_mul(t2, w, 0.5)
    nc.vector.tensor_sub(ref[:, :, 1], cx, t2)
    nc.vector.tensor_add(ref[:, :, 3], cx, t2)
    nc.vector.tensor_scalar(
        out=ref, in0=ref, scalar1=0.0, scalar2=1.0,
        op0=mybir.AluOpType.max, op1=mybir.AluOpType.min,
    )

    # ---- NMS state ----
    # valid = (cid > 0) & (score >= MIN_CONF); sc0 = valid ? score : NEG
    vf = sm.tile([P, NPI], F32, tag=f"vf{img}")
    nc.vector.tensor_single_scalar(vf, cid, 0.5, op=mybir.AluOpType.is_ge)
    v = sm.tile([P, NPI], mybir.dt.uint8, tag=f"v{img}")
    nc.vector.tensor_mul(v, vf, ge)
    nc.vector.tensor_copy(sc, negs)
    nc.vector.copy_predicated(sc, v, scores)

    # offset boxes = ref + 2*cid, per-class NMS trick
    nc.vector.scalar_tensor_tensor(
        out=ob, in0=cid.unsqueeze(2).to_broadcast([P, NPI, 4]), scalar=2.0,
        in1=ref, op0=mybir.AluOpType.mult, op1=mybir.AluOpType.add,
    )
    # areas of offset boxes
    ar2 = sm.tile([P, NPI, 2], F32, tag=f"ar2{img}")
    nc.vector.tensor_sub(ar2, ob[:, :, 2:4], ob[:, :, 0:2])
    nc.vector.tensor_mul(ar, ar2[:, :, 0], ar2[:, :, 1])
    # cat = [ref(4), cid, score] for one-shot row extraction
    nc.vector.tensor_copy(cat[:, :, 0:4], ref)
    nc.vector.tensor_copy(cat[:, :, 4], cid)
    nc.vector.tensor_copy(cat[:, :, 5], scores)
    nc.vector.memset(mr, NEG)

    state[f"sc{img}"] = sc
    state[f"ob{img}"] = ob
    state[f"ar{img}"] = ar
    state[f"cat{img}"] = cat
    state[f"mr{img}"] = mr


def _nms_loop(nc, tc, pools, img, state, rv, det):
    """T = min(100, count) NMS iterations for one image."""
    cpool, big, sm = pools
    sc = state[f"sc{img}"]
    ob = state[f"ob{img}"]
    ar = state[f"ar{img}"]
    cat = state[f"cat{img}"]
    mr = state[f"mr{img}"]
    negs = state["negs"]

    t_end = smin(rv, K)
    with tc.For_i(0, t_end, name=f"nms{img}") as i:
        pm = sm.tile([P, 1], F32, tag=f"pm{img}")
        nc.vector.reduce_max(pm, sc, axis=mybir.AxisListType.X)
        gm = sm.tile([P, 1], F32, tag=f"gm{img}")
        nc.gpsimd.partition_all_reduce(gm, pm, channels=P,
                                       reduce_op=bass_isa.ReduceOp.max)
        # mask of selected candidate
        msk = sm.tile([P, NPI], F32, tag=f"msk{img}")
        nc.vector.tensor_tensor(msk, sc, gm.to_broadcast([P, NPI]),
                                op=mybir.AluOpType.is_equal)
        # extract its [ref, cid, score] row via masked sum
        mb6 = sm.tile([P, NPI, 6], F32, tag=f"mb6{img}")
        nc.vector.tensor_tensor(
            mb6, cat, msk.unsqueeze(2).to_broadcast([P, NPI, 6]),
            op=mybir.AluOpType.mult,
        )
        r6p = sm.tile([P, 6], F32, tag=f"r6p{img}")
        nc.vector.reduce_sum(r6p, mb6.rearrange("p n k -> p k n"),
                             axis=mybir.AxisListType.X)
        r6 = sm.tile([P, 6], F32, tag=f"r6{img}")
        nc.gpsimd.partition_all_reduce(r6, r6p, channels=P,
                                       reduce_op=bass_isa.ReduceOp.add)
        # zero the row when scores are exhausted (gm == NEG)
        okm = sm.tile([P, 1], F32, tag=f"okm{img}")
        nc.vector.tensor_single_scalar(okm, gm, NEG * 0.5,
                                       op=mybir.AluOpType.is_gt)
        nc.vector.tensor_mul(r6, r6, okm.to_broadcast([P, 6]))
        nc.vector.tensor_copy(det[0:1, bass.ds(img * K * 6 + i * 6, 6)],
                              r6[0:1, :])

        # selected offset box, replicated on all partitions
        sb = sm.tile([P, 4], F32, tag=f"sb{img}")
        nc.vector.scalar_tensor_tensor(
            out=sb, in0=r6[:, 4:5].to_broadcast([P, 4]), scalar=2.0,
            in1=r6[:, 0:4], op0=mybir.AluOpType.mult, op1=mybir.AluOpType.add,
        )
        # IoU(selected, all) on offset boxes
        mx = sm.tile([P, NPI, 2], F32, tag=f"mx{img}")
        nc.vector.tensor_tensor(
            mx, ob[:, :, 0:2], sb[:, 0:2].unsqueeze(1).to_broadcast([P, NPI, 2]),
            op=mybir.AluOpType.max,
        )
        mn = sm.tile([P, NPI, 2], F32, tag=f"mn{img}")
        nc.vector.tensor_tensor(
            mn, ob[:, :, 2:4], sb[:, 2:4].unsqueeze(1).to_broadcast([P, NPI, 2]),
            op=mybir.AluOpType.min,
        )
        nc.vector.tensor_sub(mn, mn, mx)
        nc.vector.tensor_scalar_max(mn, mn, 0.0)
        inter = sm.tile([P, NPI], F32, tag=f"inter{img}")
        nc.vector.tensor_mul(inter, mn[:, :, 0], mn[:, :, 1])
        aa2 = sm.tile([P, 2], F32, tag=f"aa2{img}")
        nc.vector.tensor_sub(aa2, sb[:, 2:4], sb[:, 0:2])
        aa = sm.tile([P, 1], F32, tag=f"aa{img}")
        nc.vector.tensor_mul(aa, aa2[:, 0:1], aa2[:, 1:2])
        # suppress iff 0.3 * union < inter  (union = area_sel + areas - inter)
        u = sm.tile([P, NPI], F32, tag=f"u{img}")
        nc.vector.scalar_tensor_tensor(
            out=u, in0=ar, scalar=aa[:, 0:1], in1=inter,
            op0=mybir.AluOpType.add, op1=mybir.AluOpType.subtract,
        )
        sup = sm.tile([P, NPI], mybir.dt.uint8, tag=f"sup{img}")
        nc.vector.scalar_tensor_tensor(
            out=sup, in0=u, scalar=NMS_T, in1=inter,
            op0=mybir.AluOpType.mult, op1=mybir.AluOpType.is_lt,
        )
        nc.vector.copy_predicated(sc, sup, negs)
        # kill the selected entry itself (covers zero-area self-IoU)
        nc.vector.tensor_copy(mr[:, 0:1], gm)
        nc.vector.match_replace(out=sc, in_to_replace=mr, in_values=sc,
                                imm_value=NEG)


def build_nc():
    nc = bacc.Bacc("TRN2", target_bir_lowering=False)
    rois_t = nc.dram_tensor("rois", [BPC, N, 4], F32, kind="ExternalInput")
    probs_t = nc.dram_tensor("probs", [BPC, N, C], F32, kind="ExternalInput")
    deltas_t = nc.dram_tensor("deltas", [BPC, N, C, 4], F32, kind="ExternalInput")
    out_t = nc.dram_tensor("out", [BPC, K, 6], F32, kind="ExternalOutput")

    with TileContext(nc) as tc:
        with (
            tc.tile_pool(name="const", bufs=1) as cpool,
            tc.tile_pool(name="big", bufs=1) as big,
            tc.tile_pool(name="small", bufs=1) as sm,
            tc.tile_pool(name="psum", bufs=1, space="PSUM") as pp,
        ):
            pools = (cpool, big, sm)

            # ---------------- fast phase (always runs) ----------------
            # fused probs view: partition = img*64 + p0, 64 rois/partition.
            # Two n-halves, each DMAed as two partition-halves on the two
            # HWDGE queues: round 1 (n-half 0, both partition halves) lands
            # ~4us before round 2, so the any-detection below overlaps DMA.
            pap = probs_t.rearrange("b (p0 m) c -> (b p0) m c", m=NPF)
            engines = {"s": nc.sync, "a": nc.scalar, "g": nc.gpsimd}
            # chunk schedule: QCFG like "s32,a32" = engine + slot count per
            # chunk. Full 128-partition transfers (all 16 SDMA engines);
            # alternating the two HWDGE queues lets engines interleave both
            # queues' packets, hiding per-descriptor HBM latency.
            chunks = []
            o = 0
            for part in QCFG.split(","):
                e, n = part[0], int(part[1:])
                chunks.append((e, o, n))
                o += n
            assert o == NPF

            # guard detector: is ANY prob >= MIN_CONF?  Per chunk, work is
            # split three ways by engine throughput: DVE reduce_max on slots
            # 0:13, ACT Relu(p - (MIN_CONF - eps)) with sum-accum on 13:26
            # (in-place; eps makes p == MIN_CONF detectable), GpSimd
            # reduce_max on 26:n. pos columns are all >= 0 and positive iff
            # a prob passed.
            nch = len(chunks)
            pos = sm.tile([P, nch + 1], F32, tag="pos")
            nsc = sum(n // 2 for _, _, n in chunks)
            scores = sm.tile([P, nsc], F32, tag="scores_f")
            nbias = cpool.tile([P, 1], F32, tag="nbias")
            nc.gpsimd.memset(nbias, -(MIN_CONF - 1e-5))
            so = 0
            for ci, (e, o, n) in enumerate(chunks):
                t = big.tile([P, n, C], F32, tag=f"pfast{ci}")
                engines[e].dma_start(out=t, in_=pap[:, o : o + n])
                hd = n // 2
                nc.vector.reduce_max(
                    scores[:, so : so + hd], t[:, 0:hd],
                    axis=mybir.AxisListType.X,
                )
                so += hd
                nc.scalar.activation(
                    t[:, hd:n], t[:, hd:n],
                    mybir.ActivationFunctionType.Relu,
                    bias=nbias[:, 0:1],
                    accum_out=pos[:, ci : ci + 1],
                )
            assert so == nsc
            nc.vector.tensor_scalar(
                out=scores, in0=scores, scalar1=MIN_CONF, scalar2=None,
                op0=mybir.AluOpType.is_ge, op1=mybir.AluOpType.add,
                accum_out=pos[:, nch : nch + 1],
            )
            ones = cpool.tile([P, 1], F32, tag="ones")
            nc.gpsimd.memset(ones, 1.0)
            cntp = pp.tile([1, nch + 1], F32, tag="cntp")
            nc.tensor.matmul(cntp, ones, pos, start=True, stop=True)
            cnts = sm.tile([1, 1], F32, tag="cnts")
            nc.vector.reduce_sum(cnts, cntp, axis=mybir.AxisListType.X)
            # positive f32 total <=> positive i32 bit pattern
            rvt = nc.values_load(cnts[0:1, 0:1].bitcast(I32),
                                 min_val=-(2**31), max_val=2**31 - 1,
                                 skip_runtime_bounds_check=True)

            # zeroed output, written early (overlaps the probs DMA); the
            # slow path rewrites it on the same queue (FIFO) when needed
            det = sm.tile([1, BPC * K * 6], F32, tag="det")
            nc.vector.memset(det, 0.0)
            out_ap = out_t.rearrange("b k s -> (b k s)").unsqueeze(0)
            nc.sync.dma_start(out=out_ap, in_=det)

            # ---------------- slow phase (count > 0 only) ----------------
            with tc.If(rvt > 0, name="slow"):
                crev = cpool.tile([P, NPI, C], F32, tag="crev")
                nc.gpsimd.iota(crev, pattern=[[0, NPI], [-1, C]], base=C - 1,
                               channel_multiplier=0,
                               allow_small_or_imprecise_dtypes=True)
                negs = cpool.tile([P, NPI], F32, tag="negs")
                nc.gpsimd.memset(negs, NEG)
                cnt2 = sm.tile([P, BPC], F32, tag="cnt2")
                state = {"crev": crev, "negs": negs, "cnt2": cnt2}

                for img in range(BPC):
                    _slow_image(nc, tc, pools, img, rois_t, probs_t,
                                deltas_t, state, det)

                # exact per-image counts across partitions via PE matmul
                cnt2p = pp.tile([1, BPC], F32, tag="cnt2p")
                nc.tensor.matmul(cnt2p, ones, cnt2, start=True, stop=True)
                cnt2i = sm.tile([1, BPC], I32, tag="cnt2i")
                nc.vector.tensor_copy(cnt2i, cnt2p)

                for img in range(BPC):
                    rv = nc.values_load(cnt2i[0:1, img : img + 1],
                                        min_val=0, max_val=N,
                                        skip_runtime_bounds_check=True)
                    _nms_loop(nc, tc, pools, img, state, rv, det)

                nc.sync.dma_start(out=out_ap, in_=det)
    nc.compile()
    return nc


LAST_RESULTS = None  # BassKernelResults of the most recent kernel() call


def kernel(rois, probs, deltas):
    global LAST_RESULTS
    from concourse import bass_utils

    nc = build_nc()
    in_maps = []
    for c in range(NCORES):
        sl = slice(c * BPC, (c + 1) * BPC)
        in_maps.append({
            "rois": np.ascontiguousarray(rois[sl], dtype=np.float32),
            "probs": np.ascontiguousarray(probs[sl], dtype=np.float32),
            "deltas": np.ascontiguousarray(deltas[sl], dtype=np.float32),
        })
    res = bass_utils.run_bass_kernel_spmd(nc, in_maps, core_ids=list(range(NCORES)))
    LAST_RESULTS = res
    return np.concatenate([r["out"] for r in res.results], axis=0)


if __name__ == "__main__":
    rng = np.random.default_rng(0)
    out = kernel(
        rng.random((B, N, 4), np.float32),
        rng.random((B, N, C), np.float32),
        rng.standard_normal((B, N, C, 4)).astype(np.float32),
    )
    print(out.shape, np.abs(out).max())


# revision 24
# speedup vs baseline: 1.0613x; 1.0613x over previous
"""Trainium2 Bass kernel for DetectionLayer (refine + per-class NMS).

Contract: kernel(rois, probs, deltas) with FULL inputs
  rois   [16, 4096, 4]   f32
  probs  [16, 4096, 81]  f32
  deltas [16, 4096, 81, 4] f32
returns [16, 100, 6] f32 detections, matching the jax reference.

Sharding: pure data parallel - 2 images per core across 8 NeuronCores.

Structure:
  Fast phase (always): load probs for both images in a fused
  [128, 64, 81] layout as two full-partition chunks, one per HWDGE
  queue (SDMA engines interleave both queues' packets, hiding
  per-descriptor HBM latency -> ~350 GB/s). Detect whether ANY roi
  passes min-confidence with the work split between DVE (reduce_max +
  is_ge count) and the Scalar engine (Relu(p - (conf - eps)) with
  sum-accum), reduce via a PE ones-matmul, and write a zeroed output
  early (overlaps the DMA).
  Guard: a single tc.If(any > 0) branch (a branch, not a loop skip --
  no back-edge barriers).
  Slow phase (inside the If; only when detections exist): per-image
  probs/rois/deltas load, argmax-class delta select, box refine,
  per-class NMS via the class-offset trick, rewrite of the output.

Known limitation (inherited from the original kernel, NMS loop kept
bit-identical): when two rois have exactly equal f32 class scores, the
masked-sum row extraction in the NMS loop merges both tied rows (the
reference argmax keeps only the first). Only reachable on data with
exact score ties among detections above min-confidence.
"""

import os as _os

import numpy as np

import concourse.bacc as bacc
import concourse.bass as bass
import concourse.bass_isa as bass_isa
import concourse.mybir as mybir
from concourse.expressions import smin
from concourse.tile import TileContext

B = 16              # full batch
NCORES = 8
BPC = B // NCORES   # images per core
N = 4096            # rois per image
C = 81              # classes
K = 100             # detection_max_instances
P = 128             # SBUF partitions
NPF = BPC * N // P  # fused rois per partition (64); partition p -> image p//64
NPI = N // P        # rois per partition, per-image layout (32)
NEG = -1e9
MIN_CONF = 0.7
NMS_T = 0.3
F32 = mybir.dt.float32
I32 = mybir.dt.int32

# probs DMA chunk schedule: <engine><slots>,... with s=sync, a=scalar(ACT)
QCFG = _os.environ.get("DETK_Q", "s32,a16,a16")


def _slow_image(nc, tc, pools, img, rois_t, probs_t, deltas_t, state, det):
    """Refine + NMS-state for one image (runs only inside the If guard).
    Re-loads probs in the per-image layout and recomputes scores there."""
    cpool, big, sm = pools
    crev = state["crev"]
    negs = state["negs"]
    cnt2 = state["cnt2"]

    probs_ap = probs_t[img].rearrange("(p n) c -> p n c", p=P)        # [128,32,81]
    rois_ap = rois_t[img].rearrange("(p n) k -> p n k", p=P)          # [128,32,4]
    deltas_ap = deltas_t[img].rearrange("(p n) c k -> p n c k", p=P)

    pt = big.tile([P, NPI, C], F32, tag=f"probs{img}")
    for s in range(4):
        sl = slice(32 * s, 32 * s + 32)
        nc.sync.dma_start(out=pt[sl], in_=probs_ap[sl])
    dt_ = big.tile([P, NPI, C, 4], F32, tag=f"deltas{img}")
    for s in range(8):
        sl = slice(16 * s, 16 * s + 16)
        nc.sync.dma_start(out=dt_[sl], in_=deltas_ap[sl])
    rt = sm.tile([P, NPI, 4], F32, tag=f"rois{img}")
    nc.sync.dma_start(out=rt, in_=rois_ap)

    scores = sm.tile([P, NPI], F32, tag=f"sc_s{img}")
    nc.vector.reduce_max(scores, pt, axis=mybir.AxisListType.X)
    ge = sm.tile([P, NPI], F32, tag=f"ge{img}")
    nc.vector.tensor_scalar(
        out=ge, in0=scores, scalar1=MIN_CONF, scalar2=None,
        op0=mybir.AluOpType.is_ge, op1=mybir.AluOpType.add,
        accum_out=cnt2[:, img : img + 1],
    )

    # NMS-state tiles read by the NMS loop
    sc = sm.tile([P, NPI], F32, tag=f"sc{img}")
    ob = sm.tile([P, NPI, 4], F32, tag=f"ob{img}")
    ar = sm.tile([P, NPI], F32, tag=f"ar{img}")
    cat = sm.tile([P, NPI, 6], F32, tag=f"cat{img}")
    mr = sm.tile([P, 8], F32, tag=f"mr{img}")

    # one-hot mask of argmax class: M = (probs == score), in place over probs
    m = pt
    nc.vector.tensor_tensor(
        m, pt, scores.unsqueeze(2).to_broadcast([P, NPI, C]),
        op=mybir.AluOpType.is_equal,
    )

    # select argmax-class delta: deltas *= M (bcast over k), sum over c
    d_perm = dt_.rearrange("p n c k -> p n k c")
    nc.vector.tensor_tensor(
        d_perm, d_perm, m.unsqueeze(2).to_broadcast([P, NPI, 4, C]),
        op=mybir.AluOpType.mult,
    )
    dsel = sm.tile([P, NPI, 4], F32, tag=f"dsel{img}")
    nc.vector.reduce_sum(dsel, d_perm, axis=mybir.AxisListType.X)

    # class id = 80 - max((80-c) * M)  (ties -> smallest c, like argmax)
    nc.vector.tensor_tensor(m, m, crev, op=mybir.AluOpType.mult)
    cid = sm.tile([P, NPI], F32, tag=f"cid{img}")
    nc.vector.reduce_max(cid, m, axis=mybir.AxisListType.X)
    nc.vector.tensor_scalar(
        out=cid, in0=cid, scalar1=-1.0, scalar2=float(C - 1),
        op0=mybir.AluOpType.mult, op1=mybir.AluOpType.add,
    )

    # bbox_std scaling (match reference op order exactly)
    nc.vector.tensor_scalar_mul(dsel[:, :, 0:2], dsel[:, :, 0:2], 0.1)
    nc.vector.tensor_scalar_mul(dsel[:, :, 2:4], dsel[:, :, 2:4], 0.2)

    # ---- apply deltas + clip (mirrors _apply_deltas fp32 op order) ----
    h = sm.tile([P, NPI], F32, tag=f"h{img}")
    w = sm.tile([P, NPI], F32, tag=f"w{img}")
    nc.vector.tensor_sub(h, rt[:, :, 2], rt[:, :, 0])
    nc.vector.tensor_sub(w, rt[:, :, 3], rt[:, :, 1])
    t1 = sm.tile([P, NPI], F32, tag=f"t1{img}")
    t2 = sm.tile([P, NPI], F32, tag=f"t2{img}")
    cy = sm.tile([P, NPI], F32, tag=f"cy{img}")
    cx = sm.tile([P, NPI], F32, tag=f"cx{img}")
    # cy = y1 + 0.5*h + dy*h
    nc.vector.tensor_scalar_mul(t1, h, 0.5)
    nc.vector.tensor_add(t2, rt[:, :, 0], t1)
    nc.vector.tensor_mul(t1, dsel[:, :, 0], h)
    nc.vector.tensor_add(cy, t2, t1)
    # cx = x1 + 0.5*w + dx*w
    nc.vector.tensor_scalar_mul(t1, w, 0.5)
    nc.vector.tensor_add(t2, rt[:, :, 1], t1)
    nc.vector.tensor_mul(t1, dsel[:, :, 1], w)
    nc.vector.tensor_add(cx, t2, t1)
    # h *= exp(dh); w *= exp(dw)
    e = sm.tile([P, NPI], F32, tag=f"e{img}")
    nc.scalar.activation(e, dsel[:, :, 2], mybir.ActivationFunctionType.Exp)
    nc.vector.tensor_mul(h, h, e)
    nc.scalar.activation(e, dsel[:, :, 3], mybir.ActivationFunctionType.Exp)
    nc.vector.tensor_mul(w, w, e)

    ref = sm.tile([P, NPI, 4], F32, tag=f"ref{img}")
    nc.vector.tensor_scalar_mul(t1, h, 0.5)
    nc.vector.tensor_sub(ref[:, :, 0], cy, t1)
    nc.vector.tensor_add(ref[:, :, 2], cy, t1)
    nc.vector.tensor_scalar_mul(t2, w, 0.5)
    nc.vector.tensor_sub(ref[:, :, 1], cx, t2)
    nc.vector.tensor_add(ref[:, :, 3], cx, t2)
    nc.vector.tensor_scalar(
        out=ref, in0=ref, scalar1=0.0, scalar2=1.0,
        op0=mybir.AluOpType.max, op1=mybir.AluOpType.min,
    )

    # ---- NMS state ----
    # valid = (cid > 0) & (score >= MIN_CONF); sc0 = valid ? score : NEG
    vf = sm.tile([P, NPI], F32, tag=f"vf{img}")
    nc.vector.tensor_single_scalar(vf, cid, 0.5, op=mybir.AluOpType.is_ge)
    v = sm.tile([P, NPI], mybir.dt.uint8, tag=f"v{img}")
    nc.vector.tensor_mul(v, vf, ge)
    nc.vector.tensor_copy(sc, negs)
    nc.vector.copy_predicated(sc, v, scores)

    # offset boxes = ref + 2*cid, per-class NMS trick
    nc.vector.scalar_tensor_tensor(
        out=ob, in0=cid.unsqueeze(2).to_broadcast([P, NPI, 4]), scalar=2.0,
        in1=ref, op0=mybir.AluOpType.mult, op1=mybir.AluOpType.add,
    )
    # areas of offset boxes
    ar2 = sm.tile([P, NPI, 2], F32, tag=f"ar2{img}")
    nc.vector.tensor_sub(ar2, ob[:, :, 2:4], ob[:, :, 0:2])
    nc.vector.tensor_mul(ar, ar2[:, :, 0], ar2[:, :, 1])
    # cat = [ref(4), cid, score] for one-shot row extraction
    nc.vector.tensor_copy(cat[:, :, 0:4], ref)
    nc.vector.tensor_copy(cat[:, :, 4], cid)
    nc.vector.tensor_copy(cat[:, :, 5], scores)
    nc.vector.memset(mr, NEG)

    state[f"sc{img}"] = sc
    state[f"ob{img}"] = ob
    state[f"ar{img}"] = ar
    state[f"cat{img}"] = cat
    state[f"mr{img}"] = mr


def _nms_loop(nc, tc, pools, img, state, rv, det):
    """T = min(100, count) NMS iterations for one image."""
    cpool, big, sm = pools
    sc = state[f"sc{img}"]
    ob = state[f"ob{img}"]
    ar = state[f"ar{img}"]
    cat = state[f"cat{img}"]
    mr = state[f"mr{img}"]
    negs = state["negs"]

    t_end = smin(rv, K)
    with tc.For_i(0, t_end, name=f"nms{img}") as i:
        pm = sm.tile([P, 1], F32, tag=f"pm{img}")
        nc.vector.reduce_max(pm, sc, axis=mybir.AxisListType.X)
        gm = sm.tile([P, 1], F32, tag=f"gm{img}")
        nc.gpsimd.partition_all_reduce(gm, pm, channels=P,
                                       reduce_op=bass_isa.ReduceOp.max)
        # mask of selected candidate
        msk = sm.tile([P, NPI], F32, tag=f"msk{img}")
        nc.vector.tensor_tensor(msk, sc, gm.to_broadcast([P, NPI]),
                                op=mybir.AluOpType.is_equal)
        # extract its [ref, cid, score] row via masked sum
        mb6 = sm.tile([P, NPI, 6], F32, tag=f"mb6{img}")
        nc.vector.tensor_tensor(
            mb6, cat, msk.unsqueeze(2).to_broadcast([P, NPI, 6]),
            op=mybir.AluOpType.mult,
        )
        r6p = sm.tile([P, 6], F32, tag=f"r6p{img}")
        nc.vector.reduce_sum(r6p, mb6.rearrange("p n k -> p k n"),
                             axis=mybir.AxisListType.X)
        r6 = sm.tile([P, 6], F32, tag=f"r6{img}")
        nc.gpsimd.partition_all_reduce(r6, r6p, channels=P,
                                       reduce_op=bass_isa.ReduceOp.add)
        # zero the row when scores are exhausted (gm == NEG)
        okm = sm.tile([P, 1], F32, tag=f"okm{img}")
        nc.vector.tensor_single_scalar(okm, gm, NEG * 0.5,
                                       op=mybir.AluOpType.is_gt)
        nc.vector.tensor_mul(r6, r6, okm.to_broadcast([P, 6]))
        nc.vector.tensor_copy(det[0:1, bass.ds(img * K * 6 + i * 6, 6)],
                              r6[0:1, :])

        # selected offset box, replicated on all partitions
        sb = sm.tile([P, 4], F32, tag=f"sb{img}")
        nc.vector.scalar_tensor_tensor(
            out=sb, in0=r6[:, 4:5].to_broadcast([P, 4]), scalar=2.0,
            in1=r6[:, 0:4], op0=mybir.AluOpType.mult, op1=mybir.AluOpType.add,
        )
        # IoU(selected, all) on offset boxes
        mx = sm.tile([P, NPI, 2], F32, tag=f"mx{img}")
        nc.vector.tensor_tensor(
            mx, ob[:, :, 0:2], sb[:, 0:2].unsqueeze(1).to_broadcast([P, NPI, 2]),
            op=mybir.AluOpType.max,
        )
        mn = sm.tile([P, NPI, 2], F32, tag=f"mn{img}")
        nc.vector.tensor_tensor(
            mn, ob[:, :, 2:4], sb[:, 2:4].unsqueeze(1).to_broadcast([P, NPI, 2]),
            op=mybir.AluOpType.min,
        )
        nc.vector.tensor_sub(mn, mn, mx)
        nc.vector.tensor_scalar_max(mn, mn, 0.0)
        inter = sm.tile([P, NPI], F32, tag=f"inter{img}")
        nc.vector.tensor_mul(inter, mn[:, :, 0], mn[:, :, 1])
        aa2 = sm.tile([P, 2], F32, tag=f"aa2{img}")
        nc.vector.tensor_sub(aa2, sb[:, 2:4], sb[:, 0:2])
        aa = sm.tile([P, 1], F32, tag=f"aa{img}")
        nc.vector.tensor_mul(aa, aa2[:, 0:1], aa2[:, 1:2])
        # suppress iff 0.3 * union < inter  (union = area_sel + areas - inter)
        u = sm.tile([P, NPI], F32, tag=f"u{img}")
        nc.vector.scalar_tensor_tensor(
            out=u, in0=ar, scalar=aa[:, 0:1], in1=inter,
            op0=mybir.AluOpType.add, op1=mybir.AluOpType.subtract,
        )
        sup = sm.tile([P, NPI], mybir.dt.uint8, tag=f"sup{img}")
        nc.vector.scalar_tensor_tensor(
            out=sup, in0=u, scalar=NMS_T, in1=inter,
            op0=mybir.AluOpType.mult, op1=mybir.AluOpType.is_lt,
        )
        nc.vector.copy_predicated(sc, sup, negs)
        # kill the selected entry itself (covers zero-area self-IoU)
        nc.vector.tensor_copy(mr[:, 0:1], gm)
        nc.vector.match_replace(out=sc, in_to_replace=mr, in_values=sc,
                                imm_value=NEG)


def build_nc():
    nc = bacc.Bacc("TRN2", target_bir_lowering=False)
    rois_t = nc.dram_tensor("rois", [BPC, N, 4], F32, kind="ExternalInput")
    probs_t = nc.dram_tensor("probs", [BPC, N, C], F32, kind="ExternalInput")
    deltas_t = nc.dram_tensor("deltas", [BPC, N, C, 4], F32, kind="ExternalInput")
    out_t = nc.dram_tensor("out", [BPC, K, 6], F32, kind="ExternalOutput")

    with TileContext(nc) as tc:
        with (
            tc.tile_pool(name="const", bufs=1) as cpool,
            tc.tile_pool(name="big", bufs=1) as big,
            tc.tile_pool(name="small", bufs=1) as sm,
            tc.tile_pool(name="psum", bufs=1, space="PSUM") as pp,
        ):
            pools = (cpool, big, sm)

            # ---------------- fast phase (always runs) ----------------
            # fused probs view: partition = img*64 + p0, 64 rois/partition.
            # Two n-halves, each DMAed as two partition-halves on the two
            # HWDGE queues: round 1 (n-half 0, both partition halves) lands
            # ~4us before round 2, so the any-detection below overlaps DMA.
            pap = probs_t.rearrange("b (p0 m) c -> (b p0) m c", m=NPF)
            engines = {"s": nc.sync, "a": nc.scalar, "g": nc.gpsimd}
            # chunk schedule: QCFG like "s32,a32" = engine + slot count per
            # chunk. Full 128-partition transfers (all 16 SDMA engines);
            # alternating the two HWDGE queues lets engines interleave both
            # queues' packets, hiding per-descriptor HBM latency.
            chunks = []
            o = 0
            for part in QCFG.split(","):
                e, n = part[0], int(part[1:])
                chunks.append((e, o, n))
                o += n
            assert o == NPF

            # guard detector: is ANY prob >= MIN_CONF?  Per chunk, work is
            # split three ways by engine throughput: DVE reduce_max on slots
            # 0:13, ACT Relu(p - (MIN_CONF - eps)) with sum-accum on 13:26
            # (in-place; eps makes p == MIN_CONF detectable), GpSimd
            # reduce_max on 26:n. pos columns are all >= 0 and positive iff
            # a prob passed.
            nch = len(chunks)
            pos = sm.tile([P, nch + 1], F32, tag="pos")
            nsc = sum(n // 2 for _, _, n in chunks)
            scores = sm.tile([P, nsc], F32, tag="scores_f")
            nbias = cpool.tile([P, 1], F32, tag="nbias")
            nc.gpsimd.memset(nbias, -(MIN_CONF - 1e-5))
            so = 0
            for ci, (e, o, n) in enumerate(chunks):
                t = big.tile([P, n, C], F32, tag=f"pfast{ci}")
                engines[e].dma_start(out=t, in_=pap[:, o : o + n])
                hd = n // 2
                nc.vector.reduce_max(
                    scores[:, so : so + hd], t[:, 0:hd],
                    axis=mybir.AxisListType.X,
                )
                so += hd
                nc.scalar.activation(
                    t[:, hd:n], t[:, hd:n],
                    mybir.ActivationFunctionType.Relu,
                    bias=nbias[:, 0:1],
                    accum_out=pos[:, ci : ci + 1],
                )
            assert so == nsc
            nc.vector.tensor_scalar(
                out=scores, in0=scores, scalar1=MIN_CONF, scalar2=None,
                op0=mybir.AluOpType.is_ge, op1=mybir.AluOpType.add,
                accum_out=pos[:, nch : nch + 1],
            )
            ones = cpool.tile([P, 1], F32, tag="ones")
            nc.gpsimd.memset(ones, 1.0)
            cntp = pp.tile([1, nch + 1], F32, tag="cntp")
            nc.tensor.matmul(cntp, ones, pos, start=True, stop=True)
            cnts = sm.tile([1, 1], F32, tag="cnts")
            nc.vector.reduce_sum(cnts, cntp, axis=mybir.AxisListType.X)
            # positive f32 total <=> positive i32 bit pattern
            rvt = nc.values_load(cnts[0:1, 0:1].bitcast(I32),
                                 min_val=-(2**31), max_val=2**31 - 1,
                                 skip_runtime_bounds_check=True)

            # zeroed output, written early (overlaps the probs DMA); the
            # slow path rewrites it on the same queue (FIFO) when needed
            det = sm.tile([1, BPC * K * 6], F32, tag="det")
            nc.vector.memset(det, 0.0)
            out_ap = out_t.rearrange("b k s -> (b k s)").unsqueeze(0)
            nc.sync.dma_start(out=out_ap, in_=det)

            # ---------------- slow phase (count > 0 only) ----------------
            with tc.If(rvt > 0, name="slow"):
                crev = cpool.tile([P, NPI, C], F32, tag="crev")
                nc.gpsimd.iota(crev, pattern=[[0, NPI], [-1, C]], base=C - 1,
                               channel_multiplier=0,
                               allow_small_or_imprecise_dtypes=True)
                negs = cpool.tile([P, NPI], F32, tag="negs")
                nc.gpsimd.memset(negs, NEG)
                cnt2 = sm.tile([P, BPC], F32, tag="cnt2")
                state = {"crev": crev, "negs": negs, "cnt2": cnt2}

                for img in range(BPC):
                    _slow_image(nc, tc, pools, img, rois_t, probs_t,
                                deltas_t, state, det)

                # exact per-image counts across partitions via PE matmul
                cnt2p = pp.tile([1, BPC], F32, tag="cnt2p")
                nc.tensor.matmul(cnt2p, ones, cnt2, start=True, stop=True)
                cnt2i = sm.tile([1, BPC], I32, tag="cnt2i")
                nc.vector.tensor_copy(cnt2i, cnt2p)

                for img in range(BPC):
                    rv = nc.values_load(cnt2i[0:1, img : img + 1],
                                        min_val=0, max_val=N,
                                        skip_runtime_bounds_check=True)
                    _nms_loop(nc, tc, pools, img, state, rv, det)

                nc.sync.dma_start(out=out_ap, in_=det)
    nc.compile()
    return nc


LAST_RESULTS = None  # BassKernelResults of the most recent kernel() call


def kernel(rois, probs, deltas):
    global LAST_RESULTS
    from concourse import bass_utils

    nc = build_nc()
    in_maps = []
    for c in range(NCORES):
        sl = slice(c * BPC, (c + 1) * BPC)
        in_maps.append({
            "rois": np.ascontiguousarray(rois[sl], dtype=np.float32),
            "probs": np.ascontiguousarray(probs[sl], dtype=np.float32),
            "deltas": np.ascontiguousarray(deltas[sl], dtype=np.float32),
        })
    res = bass_utils.run_bass_kernel_spmd(nc, in_maps, core_ids=list(range(NCORES)))
    LAST_RESULTS = res
    return np.concatenate([r["out"] for r in res.results], axis=0)


if __name__ == "__main__":
    rng = np.random.default_rng(0)
    out = kernel(
        rng.random((B, N, 4), np.float32),
        rng.random((B, N, C), np.float32),
        rng.standard_normal((B, N, C, 4)).astype(np.float32),
    )
    print(out.shape, np.abs(out).max())
